# revision 14
# baseline (speedup 1.0000x reference)
"""KAN layer kernel for 8 Trainium2 NeuronCores.

Math (reference):
    basis[b,i] = sum_h silu(x[b,i]*w1[i%K,h] + b1[i%K,h]) * w2[i%K,h] + b2[i%K]
    out[b,o]   = sum_i basis[b,i] * Wsum[o,i],   Wsum = W.sum(-1)   # [O,I]

Strategy (memory-bound on streaming W; per-core ~21 MB of bf16):
  - Features are permuted so they are sorted by k = i%K.  Each SBUF
    partition then holds NT features of a SINGLE k, so per-feature MLP
    params are per-partition scalars: the affine z = x*w1+b1 runs as
    4x-mode tensor_scalar ops (two per-partition scalar operands) over
    4-tile groups, silu as one wide ACT op per group, the *w2 and h-fold
    tree as 2x-mode bf16 DVE ops.  The final +b2 rides the ACT bias.
  - W is cast to bf16 on host (tolerance 2e-2, measured ~5e-3) and
    streamed with plain DMAs on two dedicated rings (sync HWDGE for even
    tiles, gpsimd SWDGE for odd).  Each ring is headed by a tiny const
    transfer (~0.5 MB total) so the basis inputs are not stuck behind
    megabytes of W on the shared SDMA engines.  The K-reduction rides
    the PE's PSUM accumulation (170 matmuls), hidden under the DMA
    stream; 10 W buffers decouple DMA issue from the mm->recycle loop.
  - Data-parallel over features: core c takes 121 partitions x 17 slots
    of the k-sorted (padded) feature list; partial out[64,1024] summed on
    host.
"""
import numpy as np

B, I, O, K, H = 64, 16384, 1024, 5, 16
NCORES = 8
NT = 17                   # feature slots per partition (= i-tiles per core)
G = 4                     # tiles per basis group (NT = 1 solo + 4 groups)
NG = 4
GP = 193                  # partitions per k-group (ceil(3277/17))
APC = 121                 # active partitions per core (8*121=968 >= 5*193)
NPART = NCORES * APC      # 968 partitions globally
P = 128

TRACE = False             # test.py sets True to capture an NTFF profile
LAST_RESULT = None


def _build():
    from contextlib import ExitStack
    from concourse import bacc, mybir, tile

    f32 = mybir.dt.float32
    bf16 = mybir.dt.bfloat16
    AT = mybir.ActivationFunctionType
    OP = mybir.AluOpType
    nc = bacc.Bacc("TRN2", target_bir_lowering=False, debug=False,
                   num_devices=NCORES)
    Wd = nc.declare_dram_parameter("Wd", [NT, APC, K * O], bf16, isOutput=False)
    xd = nc.declare_dram_parameter("xd", [P, NT * B], bf16, isOutput=False)
    prd = nc.declare_dram_parameter("prd", [P, H * B], bf16, isOutput=False)
    # fpd: b2 [P,1] | w1 [P,H] | b1 [P,H]  (fp32)
    fpd = nc.declare_dram_parameter("fpd", [P, 1 + 2 * H], f32, isOutput=False)
    out = nc.declare_dram_parameter("out", [B, O], f32, isOutput=True)

    HB = H * B                # 1024
    GW = G * B                # 256: group row width (t,b)
    with tile.TileContext(nc) as tc, ExitStack() as ctx:
        const = ctx.enter_context(tc.tile_pool(name="const", bufs=1))
        wpool = ctx.enter_context(tc.tile_pool(name="w", bufs=12))
        zpool = ctx.enter_context(tc.tile_pool(name="z", bufs=2))
        spool = ctx.enter_context(tc.tile_pool(name="s", bufs=2))
        fpool = ctx.enter_context(tc.tile_pool(name="fold", bufs=2))
        apool = ctx.enter_context(tc.tile_pool(name="acc", bufs=NG + 1))
        opool = ctx.enter_context(tc.tile_pool(name="out", bufs=1))
        psum = ctx.enter_context(tc.tile_pool(name="psum", bufs=1, space="PSUM"))

        # Tiny consts head each DMA ring so basis inputs land in ~2us; the
        # W stream follows immediately on both rings.
        fpsb = const.tile([P, 1 + 2 * H], f32)
        nc.sync.dma_start(fpsb[:, :], fpd[:, :])
        xsb = const.tile([P, NT * B], bf16)
        nc.scalar.dma_start(xsb[:, :], xd[:, :])
        w2rep = const.tile([P, H * B], bf16)
        nc.sync.dma_start(w2rep[:, :], prd[:, :])
        b2v = fpsb[:, 0:1]
        w1c = fpsb[:, 1:1 + H]
        b1c = fpsb[:, 1 + H:1 + 2 * H]
        w23 = w2rep[:, :].rearrange("p (h b) -> p h b", h=H)

        wts = []
        for t in range(NT):
            wts.append(wpool.tile([APC, K * O], bf16, tag="wt",
                                  name=f"wt{t}"))
        for t in range(NT):
            eng = nc.scalar if (t % 2 == 1 and t < 10) else nc.sync
            eng.dma_start(wts[t][:, :], Wd[t])

        ps0 = psum.tile([B, 512], f32, tag="ps0")
        ps1 = psum.tile([B, 512], f32, tag="ps1")
        psh = psum.tile([1, B], f32, tag="psh")
        accs = [None] * NT   # per stream-slot: (tile_ap, col0)

        def heartbeat(src):
            # Tiny matmul on a fresh tile: keeps the HAM clock gate at
            # K=8/8 through the basis phase so real matmuls run at 2.4 GHz.
            nc.tensor.matmul(psh[:, :], src[:, 0:1], src[:, 0:B],
                             start=True, stop=True)

        def basis_group(c0, gw, name):
            """Tiles at x cols [c0, c0+gw): returns acc tile [P, gw]."""
            nt = gw // B
            xs = xsb[:, c0:c0 + gw]
            zg = zpool.tile([P, H * gw], bf16, tag=f"z{nt}", name=f"z{name}")
            z3 = zg[:, :].rearrange("p (h c) -> p h c", h=H)
            for h in range(H):
                nc.vector.tensor_scalar(
                    z3[:, h, :], xs, w1c[:, h:h + 1], b1c[:, h:h + 1],
                    op0=OP.mult, op1=OP.add)
            heartbeat(zg)
            sg = spool.tile([P, H * gw], bf16, tag=f"s{nt}", name=f"s{name}")
            nc.scalar.activation(sg[:, :], zg[:, :], AT.Silu)
            # sw = s * w2 (in place), w2 broadcast over the tile axis
            s4 = sg[:, :].rearrange("p (h j b) -> p h j b", h=H, j=nt)
            w2b = w23[:, :, None, :].to_broadcast((P, H, nt, B))
            nc.vector.tensor_mul(s4, s4, w2b)
            f8 = fpool.tile([P, 8 * gw], bf16, tag=f"f8{nt}", name=f"f8{name}")
            nc.vector.tensor_add(f8[:, :], sg[:, 0:8 * gw], sg[:, 8 * gw:16 * gw])
            f4 = fpool.tile([P, 4 * gw], bf16, tag=f"f4{nt}", name=f"f4{name}")
            nc.vector.tensor_add(f4[:, :], f8[:, 0:4 * gw], f8[:, 4 * gw:8 * gw])
            f2 = fpool.tile([P, 2 * gw], bf16, tag=f"f2{nt}", name=f"f2{name}")
            nc.vector.tensor_add(f2[:, :], f4[:, 0:2 * gw], f4[:, 2 * gw:4 * gw])
            ap = fpool.tile([P, gw], bf16, tag=f"ap{nt}", name=f"ap{name}")
            nc.vector.tensor_add(ap[:, :], f2[:, 0:gw], f2[:, gw:2 * gw])
            acc = apool.tile([P, gw], bf16, tag=f"acc{nt}", name=f"acc{name}")
            nc.scalar.activation(acc[:, :], ap[:, :], AT.Identity, bias=b2v)
            return acc

        # solo tile (stream slot 0), then 4 groups of 4
        acc0 = basis_group(0, B, "solo")
        accs[0] = (acc0, 0)
        for g in range(NG):
            accg = basis_group((1 + g * G) * B, GW, f"g{g}")
            for tg in range(G):
                accs[1 + g * G + tg] = (accg, tg * B)

        # ---- matmuls: accumulate over (t, k) on the PE ----
        for t in range(NT):
            wt = wts[t]
            heartbeat(wt)
            at, ac = accs[t]
            lhsT = at[0:APC, ac:ac + B]
            for k in range(K):
                st = (t == 0 and k == 0)
                sp = (t == NT - 1 and k == K - 1)
                nc.tensor.matmul(ps0[:, :], lhsT,
                                 wt[:, k * O:k * O + 512], start=st, stop=sp)
                nc.tensor.matmul(ps1[:, :], lhsT,
                                 wt[:, k * O + 512:(k + 1) * O], start=st, stop=sp)

        out_sb = opool.tile([B, O], f32)
        nc.scalar.copy(out_sb[:, 0:512], ps0[:, :])
        nc.vector.tensor_copy(out_sb[:, 512:1024], ps1[:, :])
        nc.sync.dma_start(out[:, :], out_sb[:, :])
    nc.compile()
    return nc


def kernel(x, w1, b1, w2, b2, W):
    global LAST_RESULT
    import ml_dtypes
    from concourse.bass_utils import run_bass_kernel_spmd

    bf16 = ml_dtypes.bfloat16
    x = np.asarray(x, dtype=np.float32)
    W = np.asarray(W, dtype=np.float32)
    w1 = np.asarray(w1, dtype=np.float32)
    b1 = np.asarray(b1, dtype=np.float32)
    w2 = np.asarray(w2, dtype=np.float32)
    b2 = np.asarray(b2, dtype=np.float32)

    # ---- k-sorted feature permutation, padded so every partition holds
    # NT features of a single k ----
    kvec = np.arange(I) % K
    order = np.argsort(kvec, kind="stable")
    counts = [int(np.sum(kvec == k)) for k in range(K)]       # 3277x4, 3276
    plist = np.full(NPART * NT, -1, dtype=np.int64)
    off = 0
    for k in range(K):
        g0 = k * GP * NT
        plist[g0:g0 + counts[k]] = order[off:off + counts[k]]
        off += counts[k]
    feats = plist.reshape(NPART, NT)                          # [968, 17]
    Fidx = np.where(feats < 0, I, feats)                      # pad -> row I
    kpart = np.minimum(np.arange(NPART) // GP, K - 1)         # k per partition

    # ---- host prep ----
    xT = np.concatenate([np.ascontiguousarray(x.T),
                         np.zeros((1, B), np.float32)])       # [I+1, B]
    WT = np.ascontiguousarray(W.reshape(O, I * K).T).reshape(I, K, O)
    WTb = np.concatenate([WT, np.zeros((1, K, O), np.float32)]).astype(bf16)

    w2rep = np.repeat(w2[kpart][:, :, None], B, axis=2).reshape(NPART, H * B)
    w1f = w1[kpart]                                           # [NPART, H]
    b1f = b1[kpart]
    b2f = b2[kpart].reshape(NPART, 1)

    in_maps = []
    for c in range(NCORES):
        rows = slice(c * APC, (c + 1) * APC)
        Fc = Fidx[rows]                                       # [121, 17]
        xg = np.zeros((P, NT * B), np.float32)
        xg[:APC] = xT[Fc].reshape(APC, NT * B)
        pr = np.zeros((P, H * B), np.float32)
        pr[:APC] = w2rep[rows]
        fp = np.zeros((P, 1 + 2 * H), np.float32)
        fp[:APC, 0:1] = b2f[rows]
        fp[:APC, 1:1 + H] = w1f[rows]
        fp[:APC, 1 + H:] = b1f[rows]
        Wc = np.ascontiguousarray(
            WTb[Fc].transpose(1, 0, 2, 3).reshape(NT, APC, K * O))
        in_maps.append({
            "Wd": Wc,
            "xd": xg.astype(bf16),
            "prd": pr.astype(bf16),
            "fpd": fp,
        })

    nc = _build()
    res = run_bass_kernel_spmd(nc, in_maps, list(range(NCORES)), trace=TRACE)
    LAST_RESULT = res
    out = np.zeros((B, O), dtype=np.float32)
    for c in range(NCORES):
        out += res.results[c]["out"]
    return out
